# revision 69
# baseline (speedup 1.0000x reference)
"""Multi-head self-attention (B=2, N=2048, D=1024, H=16) on 8 Trainium2 cores.

Sharding: core c -> batch b = c // 4, head group g = c % 4 (heads 4g..4g+3,
as two pairs).  The attention path runs in fp8 (e4m3 operands, e5m2 softmax
weights) using DoubleRow matmuls (two fp8 k-tiles per pass = 0.5 cycles/row);
accuracy survives because the residual `x` carries ~94% of the output norm
and the host combines partial projections in float64.

Score pre-conditioning is computed by the PE itself:
  sc = 5.770780 * (q.k / 8) + 32     [q scaled 0.7213475 host-side in wq;
                                      +32 via an augmented q/k row 8*4]
so the softmax weight e^z / 128 (z = q.k / 8) is EXACTLY the e5m2 bitcast of
round(clamp(sc, 0, 123)) (Schraudolph).  ACT and DVE are the only engines
that can read PSUM, so every psum evacuation flows through a greedy 2-way
balancer with per-engine cost models; each score tile [128, 2, 512] (both
heads of the pair) is evacuated by ONE 1024-wide op (ACT: true Exp with
matching scale/bias; DVE: one-op clamp-convert).

Layouts: scores use a single [128, 2, 512] psum ring (tag sc, bufs=3, 6 of
8 banks) shared with the qkv/v/proj psum needs; kT/qT live in a [33, 2]-slot
DoubleRow layout built by SBUF->SBUF DMA shuffles; PV accumulates
[v | 1/64]^T e per jt-pair (row 64 = denominator/64, so its reciprocal is
the 64x-scaled normalizer directly); DVE reciprocal + a Pool
partition_broadcast (SBUF->SBUF, the one thing Pool can do) build the
per-token normalizer, and DVE multiplies produce the fp8 attnT; the
projection contracts all 256 head dims in one DoubleRow pass per output
tile, is pipelined one i-tile behind the attention blocks, and streams bf16
partials to DRAM per i-tile.  All DMA issues run on SP/Pool (and ACT/DVE
only during the idle prologue) so the two evacuation engines never stall on
descriptor generation.  Host divides by 4096 (64 attn scale * 64 wp scale)
and adds x in float64.
"""

import numpy as np
import ml_dtypes

import concourse.bass as bass
import concourse.bacc as bacc
import concourse.mybir as mybir
import concourse.tile as tile
from concourse.bass_utils import run_bass_kernel_spmd

B = 2
N = 2048
D = 1024
NH = 16
DH = 64
N_CORES = 8
TP = 4                 # head-parallel ways per batch
HPC = NH // TP         # 4 heads per core
HDIM = HPC * DH        # 256 head dims per core
PAIRS = 2

IT = 4                 # i-tiles of 512
JT = 16                # j-chunks of 128

SCH_A = 5.770780163555851      # 4*log2(e) * 8 ... b = SCH_A*z + 32
QSCALE = SCH_A / 8.0           # folded into wq on the host
ACT_SCALE = 1.0 / SCH_A * 8.0 / 8.0    # 1/5.77078
ACT_BIAS = -32.0 / SCH_A - float(np.log(128.0))

F32 = mybir.dt.float32
F32R = mybir.dt.float32r
BF16 = mybir.dt.bfloat16
F8 = mybir.dt.float8e4
F8E5 = mybir.dt.float8e5
U8 = mybir.dt.uint8
AF = mybir.ActivationFunctionType
DR = mybir.MatmulPerfMode.DoubleRow
ALU = mybir.AluOpType

E4NP = ml_dtypes.float8_e4m3


# per-op engine-busy costs (ns) from the TRN2 cost model
def c_act(cols):
    return (cols + 222) * 0.833


def c_dve(cols):
    return (cols + 120) * 1.042


class Balancer:
    """Greedy ACT/DVE load balancer for psum-evacuation ops."""

    def __init__(self):
        self.t = {"act": 0.0, "dve": 0.0}

    def pick(self, costs):
        best, best_end = None, None
        for eng, c in costs.items():
            end = self.t[eng] + c
            if best_end is None or end < best_end:
                best, best_end = eng, end
        self.t[best] += costs[best]
        return best

    def force(self, eng, cost):
        self.t[eng] += cost


def build_bass():
    nc = bacc.Bacc("TRN2", target_bir_lowering=False, debug=False)
    x_d = nc.declare_dram_parameter("xdr", [128, 4, 2, N], F8, isOutput=False)
    wq_d = nc.declare_dram_parameter("wq", [128, 4, 2, HDIM], F8, isOutput=False)
    wk_d = nc.declare_dram_parameter("wk", [128, 4, 2, HDIM], F8, isOutput=False)
    wv_d = nc.declare_dram_parameter("wv", [128, 4, 2, HDIM], F8, isOutput=False)
    wp_d = nc.declare_dram_parameter("wp", [128, 2, D], F8, isOutput=False)
    aq_d = nc.declare_dram_parameter("aug_q", [2, 2, 2, N], F8, isOutput=False)
    ak_d = nc.declare_dram_parameter("aug_k", [2, 2, 2, N], F8, isOutput=False)
    pT_d = nc.declare_dram_parameter("pT", [D, N], BF16, isOutput=True)

    bal = Balancer()

    with tile.TileContext(nc) as tc:
        with (
            tc.tile_pool(name="big", bufs=1) as big,
            tc.tile_pool(name="stage", bufs=2) as stage,
            tc.tile_pool(name="exps", bufs=2) as exps,
            tc.tile_pool(name="psum", bufs=1, space="PSUM") as psum,
        ):
            # ---- constants / inputs (order: gate-first).  Prologue DMA
            # issues spread across all queues (ACT/DVE are idle then). ----
            # qT/kT declared early so the tiny aug-row DMAs can be issued
            # first (they gate the first score matmuls)
            qT = big.tile([97, 2, 2, N], F8, tag="qT")
            kT = big.tile([97, 2, 2, N], F8, tag="kT")

            def emit_aug(p_, qeng, keng):
                for hh in range(2):
                    r = 64 * hh + 32
                    qeng.dma_start(out=qT[r:r + 1, p_, :, :],
                                   in_=aq_d[hh:hh + 1, p_, :, :])
                    keng.dma_start(out=kT[r:r + 1, p_, :, :],
                                   in_=ak_d[hh:hh + 1, p_, :, :])
            ws = {}
            for nm, src in (("k", wk_d), ("q", wq_d), ("v", wv_d)):
                t = big.tile([128, 4, 2, HDIM], F8, tag=f"w{nm}")
                eng = nc.scalar if nm in ("k", "q") else nc.sync
                eng.dma_start(out=t, in_=src[:, :, :, :])
                ws[nm] = t
            # x in token halves: half 0 feeds the first qk chains sooner.
            # SP stays free for the qT/kT shuffle DMAs on the critical path.
            xs = big.tile([128, 4, 2, N], F8, tag="xs")
            xq0 = {(0, 0): nc.sync, (0, 1): nc.scalar, (2, 0): nc.gpsimd,
                   (2, 1): nc.sync, (3, 0): nc.scalar, (3, 1): nc.gpsimd,
                   (1, 0): nc.sync, (1, 1): nc.scalar}
            xq1 = {(0, 0): nc.sync, (0, 1): nc.scalar, (2, 0): nc.gpsimd,
                   (2, 1): nc.sync, (3, 0): nc.scalar, (3, 1): nc.gpsimd,
                   (1, 0): nc.gpsimd, (1, 1): nc.gpsimd}
            for h in range(2):
                tok = slice(h * 1024, (h + 1) * 1024)
                for c, s in ((0, 0), (0, 1), (2, 0), (2, 1), (3, 0), (3, 1),
                             (1, 0), (1, 1)):
                    eng = (xq0 if h == 0 else xq1)[(c, s)]
                    eng.dma_start(out=xs[:, c, s, tok],
                                  in_=x_d[:, c, s, tok])

            recip_pad = big.tile([1, 1024], F32, tag="recip_pad")
            wps = big.tile([128, 2, D], F8, tag="wp")
            nc.gpsimd.dma_start(out=wps, in_=wp_d[:, :, :])

            emit_aug(0, nc.gpsimd, nc.scalar)

            bias_t = big.tile([128, 1], F32, tag="bias")
            nc.vector.memset(bias_t, ACT_BIAS)

            # v with trailing 1/64 column so pv row 64 = denom/64 and its
            # reciprocal is the 64x-scaled normalizer: [tok, jt, head(4), 65]
            v8 = big.tile([128, JT, 4, 80], F8, tag="v8")
            nc.gpsimd.memset(v8[:, :, :, 64:65], 1.0 / 64.0)

            attnT = big.tile([128, 2, N], F8, tag="attnT")
            bc_sb = big.tile([128, 2, 512], F32, tag="bc_sb")

            # act-table warm-up (Exp table also serves Copy)
            warm = big.tile([1, 1], F32, tag="warm")
            nc.scalar.activation(warm, bias_t[0:1, 0:1], AF.Exp)

            def evac_copy(out_ap, in_ap, cols, force=None):
                """Plain psum->sbuf cast on ACT or DVE."""
                costs = {"act": c_act(cols), "dve": c_dve(cols)}
                if force is None:
                    eng = bal.pick(costs)
                else:
                    eng = force
                    bal.force(eng, costs[eng])
                if eng == "act":
                    nc.scalar.activation(out_ap, in_ap, AF.Copy)
                else:
                    nc.vector.tensor_copy(out_ap, in_ap)

            # ---- P1 emitters (interleaved into P2 as filler units) ----
            def emit_qk(nm, dstT, p, itps=(0, 1), shuf_engs=None):
                w_s = ws[nm]
                if shuf_engs is None:
                    shuf_engs = (nc.sync, nc.sync)
                for itp in itps:  # it-pairs -> 1024 tokens each
                    st2 = stage.tile([128, 1024], F8, tag="qk_st", bufs=3)
                    ps = psum.tile([128, 2, 512], F32, tag="sc", bufs=3)
                    for ii in range(2):
                        tok = itp * 1024 + ii * 512
                        for ci, c in enumerate((0, 2, 3, 1)):
                            nc.tensor.matmul(
                                ps[:, ii, :],
                                lhsT=w_s[:, c, :, p * 128:(p + 1) * 128],
                                rhs=xs[:, c, :, tok:tok + 512],
                                start=(ci == 0),
                                stop=(ci == 3),
                                perf_mode=DR,
                            )
                    evac_copy(st2.rearrange("p (a b) -> p a b", a=2),
                              ps, 1024)
                    for hh in range(2):
                        for s in range(2):
                            r = 64 * hh + 32 * s
                            shuf_engs[hh].dma_start(
                                out=dstT[64 * hh:64 * hh + 32, p, s,
                                         itp * 1024:(itp + 1) * 1024],
                                in_=st2[r:r + 32, :],
                            )

            def emit_v(quad, force=None):
                # 4 token-tiles of 128 -> [128, 16, 64] psum -> 1024 evac
                ps = psum.tile([128, 2, 512], F32, tag="sc", bufs=3)
                psv = ps.rearrange("p a (j h d) -> p (a j) h d", h=4, d=64)
                for jj in range(4):
                    t = 4 * quad + jj
                    for c in range(4):
                        nc.tensor.matmul(
                            psv[:, jj, :, :],
                            lhsT=xs[:, c, :, t * 128:(t + 1) * 128],
                            rhs=ws["v"][:, c, :, :],
                            start=(c == 0),
                            stop=(c == 3),
                            perf_mode=DR,
                        )
                evac_copy(v8[:, 4 * quad:4 * quad + 4, :, 0:64], psv,
                          1024, force=force)

            # minimal prefix: only what the first half of block (p0, it0)
            # needs up front (kT/qT tokens 0:1024 + v quads 0-1); the rest
            # interleaves into the jt loops as psum-slot-sized filler units.
            emit_qk("k", kT, 0, itps=(0,), shuf_engs=(nc.sync, nc.scalar))
            emit_qk("q", qT, 0, itps=(0,), shuf_engs=(nc.sync, nc.scalar))
            emit_qk("k", kT, 0, itps=(1,), shuf_engs=(nc.sync, nc.scalar))
            emit_aug(1, nc.gpsimd, nc.gpsimd)
            emit_v(0, force="dve")
            emit_v(1, force="dve")
            fillq = [
                lambda: emit_v(2),
                lambda: emit_v(3),
                lambda: emit_qk("q", qT, 0, itps=(1,)),
                lambda: emit_qk("k", kT, 1, itps=(0,)),
                lambda: emit_qk("k", kT, 1, itps=(1,)),
                lambda: emit_qk("q", qT, 1, itps=(0,)),
                lambda: emit_qk("q", qT, 1, itps=(1,)),
            ]

            def emit_scores(p, it, jt):
                sc2 = psum.tile([128, 2, 512], F32, tag="sc", bufs=3)
                for hh in range(2):
                    nc.tensor.matmul(
                        sc2[:, hh, :],
                        lhsT=kT[64 * hh:64 * hh + 33, p, :,
                                jt * 128:(jt + 1) * 128],
                        rhs=qT[64 * hh:64 * hh + 33, p, :,
                               it * 512:(it + 1) * 512],
                        start=True,
                        stop=True,
                        perf_mode=DR,
                    )
                return sc2

            def emit_proj_og(it, og, force=None):
                # one output group of the projection for i-tile `it`
                pj = psum.tile([128, 2, 512], F32, tag="sc", bufs=3)
                st = stage.tile([128, 2, 512], BF16, tag="pj_st", bufs=4)
                for oo in range(2):
                    ot = 2 * og + oo
                    nc.tensor.matmul(
                        pj[:, oo, :],
                        lhsT=wps[:, :, ot * 128:(ot + 1) * 128],
                        rhs=attnT[:, :, it * 512:(it + 1) * 512],
                        start=True,
                        stop=True,
                        perf_mode=DR,
                    )
                evac_copy(st, pj, 1024, force=force)
                nc.sync.dma_start(
                    out=pT_d[og * 256:(og + 1) * 256,
                             it * 512:(it + 1) * 512].rearrange(
                                 "(a b) n -> b a n", a=2),
                    in_=st,
                )

            FILL_JTS = (5, 8, 11, 14)
            PV0 = 5   # first PV emission (pairs 0..PV0 in one burst)
            pend_mult = []
            last_eng = None   # carries across blocks (exp DD guard)

            for p in range(PAIRS):
                for it in range(IT):
                    blk_fill = []
                    if p == 0:
                        nfill = {0: 3, 1: 3, 2: 1, 3: 0}[it]
                        blk_fill = [fillq.pop(0) for _ in range(nfill)]
                    elif it >= 1:
                        blk_fill = [
                            (lambda it_=it - 1, og_=og:
                             emit_proj_og(it_, og_)) for og in range(4)]
                    e8 = exps.tile([128, JT, 2, 512], U8, tag="e8", bufs=3)
                    pv2 = psum.tile([65, 2, 512], F32, tag="pv2", bufs=1)
                    pvA = pv2[:, 0, :]
                    pvB = pv2[:, 1, :]

                    def emit_pv(jt):
                        st_, sp_ = (jt == 1), (jt == JT - 1)
                        for hh, pvx in ((0, pvA), (1, pvB)):
                            nc.tensor.matmul(
                                pvx,
                                lhsT=v8[:, jt - 1:jt + 1, 2 * p + hh, 0:65],
                                rhs=e8[:, jt - 1:jt + 1, hh, :].bitcast(F8E5),
                                start=st_,
                                stop=sp_,
                                perf_mode=DR,
                            )

                    sc_next = emit_scores(p, it, 0)
                    eng15 = None
                    for jt in range(JT):
                        sc2 = sc_next
                        if jt + 1 < JT:
                            sc_next = emit_scores(p, it, jt + 1)
                        costs = {"act": c_act(1024), "dve": c_dve(1024)}
                        if jt == JT - 2:
                            eng = bal.pick(costs)
                            eng15 = "act" if eng == "dve" else "dve"
                        elif jt == JT - 1:
                            eng = eng15
                            bal.force(eng, costs[eng])
                        elif p == 0 and it == 0 and jt == 0:
                            eng = "dve"
                            bal.force(eng, costs[eng])
                        elif last_eng == "dve":
                            # never two DVE exps in a row: a DD run blocks
                            # the score ring and idles ACT
                            eng = "act"
                            bal.force(eng, costs[eng])
                        else:
                            eng = bal.pick(costs)
                        last_eng = eng
                        e_out = e8[:, jt, :, :]
                        if eng == "act":
                            nc.scalar.activation(
                                e_out.bitcast(F8E5), sc2, AF.Exp,
                                bias=bias_t, scale=ACT_SCALE,
                            )
                        else:
                            nc.vector.tensor_scalar(
                                e_out, sc2, 123.0, 0.0,
                                ALU.min, ALU.max,
                            )
                        if jt in (0, 1, 3) and pend_mult:
                            pend_mult.pop(0)()
                        if jt == PV0:
                            for j2 in range(1, PV0 + 1, 2):
                                emit_pv(j2)
                        elif jt > PV0 and jt % 2 == 1:
                            emit_pv(jt)
                        if jt in FILL_JTS and blk_fill:
                            blk_fill.pop(0)()
                    # ---- normalization: DVE recip -> Pool partition
                    # broadcasts at block end; the two DVE multiplies are
                    # deferred into the next block's first jts so the Pool
                    # broadcast latency overlaps that block's exps ----
                    def chain_head(pv2_=pv2):
                        with nc.allow_low_precision(reason="f8 denom"):
                            nc.vector.reciprocal(
                                recip_pad[0:1, :].rearrange(
                                    "p (a b) -> p a b", a=2),
                                pv2_[64:65, :, :])
                            bal.force("dve", c_dve(1024))
                        for hh in range(2):
                            nc.gpsimd.partition_broadcast(
                                bc_sb[:, hh, :],
                                recip_pad[0:1, hh * 512:(hh + 1) * 512])
                    pend_mult.clear()
                    pend_mult.append(chain_head)
                    for hh, pvx in ((0, pvA), (1, pvB)):
                        pend_mult.append((lambda hh_=hh, pvx_=pvx, p_=p,
                                          it_=it: (
                            nc.vector.tensor_tensor(
                                attnT[64 * hh_:64 * hh_ + 64, p_,
                                      it_ * 512:(it_ + 1) * 512],
                                pvx_[0:64, :],
                                bc_sb[64 * hh_:64 * hh_ + 64, hh_, :],
                                ALU.mult),
                            bal.force("dve", c_dve(512)))))

                    if p == 1 and it == IT - 1:
                        for m in pend_mult:
                            m()
                        pend_mult.clear()
                        for og in range(4):
                            emit_proj_og(it, og)
    return nc


_NC = None


def _get_nc():
    global _NC
    if _NC is None:
        _NC = build_bass()
        _NC.finalize()
    return _NC


_AUG_Q = np.zeros((2, 2, 2, N), np.float32)
_AUG_Q[:, :, 0, :] = 8.0
_AUG_K = np.zeros((2, 2, 2, N), np.float32)
_AUG_K[:, :, 0, :] = 4.0
_AUG_Q = _AUG_Q.astype(E4NP)
_AUG_K = _AUG_K.astype(E4NP)



def _dr4(w):  # [1024, 256] -> [128, 4, 2, 256] fp8
    return np.ascontiguousarray(
        w.reshape(4, 2, 128, -1).transpose(2, 0, 1, 3).astype(E4NP))


def make_in_maps(x, w_qkv, w_proj):
    x = np.asarray(x, np.float32)
    w_qkv = np.asarray(w_qkv, np.float32)
    w_proj = np.asarray(w_proj, np.float32)
    xdrs = []
    for b in range(B):
        xT = np.ascontiguousarray(x[b].T).astype(E4NP)  # [D, N]
        xdrs.append(np.ascontiguousarray(
            xT.reshape(4, 2, 128, N).transpose(2, 0, 1, 3)))
    in_maps = []
    for c in range(N_CORES):
        b, g = divmod(c, TP)
        h0 = g * HDIM
        wp = (64.0 * w_proj[h0:h0 + HDIM, :]).astype(E4NP)  # [256, 1024]
        in_maps.append({
            "xdr": xdrs[b],
            "wq": _dr4(QSCALE * w_qkv[:, h0:h0 + HDIM]),
            "wk": _dr4(w_qkv[:, D + h0:D + h0 + HDIM]),
            "wv": _dr4(w_qkv[:, 2 * D + h0:2 * D + h0 + HDIM]),
            "wp": np.ascontiguousarray(
                wp.reshape(2, 128, D).transpose(1, 0, 2)),
            "aug_q": _AUG_Q,
            "aug_k": _AUG_K,
        })
    return in_maps


def combine_outputs(x, results):
    x = np.asarray(x, np.float32)
    out = np.empty((B, N, D), np.float32)
    for b in range(B):
        acc = x[b].astype(np.float64)
        for g in range(TP):
            pT = np.asarray(results[b * TP + g]["pT"]).astype(np.float64)
            acc += pT.T / 4096.0
        out[b] = acc.astype(np.float32)
    return out


def kernel(x, w_qkv, w_proj):
    nc = _get_nc()
    in_maps = make_in_maps(x, w_qkv, w_proj)
    res = run_bass_kernel_spmd(nc, in_maps, list(range(N_CORES))).results
    return combine_outputs(x, res)
